# revision 1
# baseline (speedup 1.0000x reference)
"""MLA (DeepSeek-style multi-head latent attention) Bass kernel for 8 trn2 NeuronCores.

Sharding: tensor-parallel over heads (2 heads/core) for the big projections +
attention; the low-rank A-projections are sequence-sharded (256 rows/core) and
the normalized latents are AllGathered in transposed [c, s] layout. The output
projection is column-parallel (each core produces 256 output channels for all
tokens) so the final combine is a host-side concat instead of an AllReduce.

All matmuls run as float32r (full fp32 storage, PE rounded mode, 1 cyc/row at
N>=256). Softmax skips max-subtraction (scores are O(+-10), exp is safe in
fp32) so the softmax denominator is a ones-matmul partition reduction.

Host-side (free) preprocessing: all weight transposes/permutations, folding
q_norm_w/kv_norm_w and SOFTMAX_SCALE into wq_b/wkv_b, rope sign folding.
"""

import math
import sys

import numpy as np

for _p in ("/opt/trn_rl_repo", "/root/.axon_site/_ro/trn_rl_repo"):
    if _p not in sys.path:
        sys.path.append(_p)

B, S, H = 1, 2048, 2048
NH = 16
Q_LORA, KV_LORA = 1536, 512
D_NOPE, D_ROPE, D_V = 128, 64, 128
D_QK = D_NOPE + D_ROPE
ROPE_FACTOR, MSCALE = 4.0, 1.0
SOFTMAX_SCALE = D_QK ** -0.5 * (0.1 * MSCALE * math.log(ROPE_FACTOR) + 1.0) ** 2
EPS = 1e-6

NCORES = 8
SSH = S // NCORES          # 256 tokens per core in stage 0
CTOT = Q_LORA + KV_LORA + D_ROPE   # 2112 latent channels
NCT = 17                   # ceil(2112/128); tile 16 only has 64 live rows

_CACHE = {}


def _build(has_mask: bool):
    import concourse.bacc as bacc
    import concourse.mybir as mybir
    import concourse.tile as tile

    f32 = mybir.dt.float32
    f32r = mybir.dt.float32r
    AF = mybir.ActivationFunctionType
    OP = mybir.AluOpType

    nc = bacc.Bacc("TRN2", target_bir_lowering=False, debug=False,
                   num_devices=NCORES)

    hidT = nc.dram_tensor("hidT", [16, 128, SSH], f32r, kind="ExternalInput")
    a_t = nc.dram_tensor("a_t", [16, 128, CTOT], f32r, kind="ExternalInput")
    cosT_sh = nc.dram_tensor("cosT_sh", [64, SSH], f32, kind="ExternalInput")
    sinTs_sh = nc.dram_tensor("sinTs_sh", [64, SSH], f32, kind="ExternalInput")
    cosT2 = nc.dram_tensor("cosT2", [128, S], f32, kind="ExternalInput")
    sinT2s = nc.dram_tensor("sinT2s", [128, S], f32, kind="ExternalInput")
    wqbT = nc.dram_tensor("wqbT", [12, 128, 384], f32r, kind="ExternalInput")
    wkvbT = nc.dram_tensor("wkvbT", [4, 128, 512], f32r, kind="ExternalInput")
    woT = nc.dram_tensor("woT", [16, 128, SSH], f32r, kind="ExternalInput")
    ones_a = nc.dram_tensor("ones_a", [128, 1], f32r, kind="ExternalInput")
    ones_b = nc.dram_tensor("ones_b", [1, 128], f32r, kind="ExternalInput")
    zer64 = nc.dram_tensor("zer64", [64, SSH], f32r, kind="ExternalInput")
    if has_mask:
        maskT = nc.dram_tensor("maskT", [S, S], f32, kind="ExternalInput")
    out = nc.dram_tensor("out", [S, SSH], f32, kind="ExternalOutput")

    bounce1 = nc.dram_tensor("bounce1", [NCT, 128, SSH], f32r)
    gath1 = nc.dram_tensor("gath1", [NCORES, NCT, 128, SSH], f32r,
                           addr_space="Shared")
    bounce2 = nc.dram_tensor("bounce2", [2, 128, S], f32r)
    gath2 = nc.dram_tensor("gath2", [16, 128, S], f32r, addr_space="Shared")

    RG = [list(range(NCORES))]

    def mm(ps, lhsT, rhs, start, stop):
        nc.tensor.matmul(ps, lhsT, rhs, start=start, stop=stop)

    from contextlib import ExitStack
    with tile.TileContext(nc) as tc, ExitStack() as _st:
        constp = _st.enter_context(tc.tile_pool(name="const", bufs=1))
        ones_col = constp.tile([128, 1], f32r)
        nc.sync.dma_start(ones_col[:], ones_a.ap())
        ones_row = constp.tile([1, 128], f32r)
        nc.sync.dma_start(ones_row[:], ones_b.ap())
        eps_sb = constp.tile([1, 1], f32)
        nc.any.memset(eps_sb[:], EPS)

        # ---------------- stage 0: latents for own 256 tokens, [c, s] layout
        with tc.tile_pool(name="s0", bufs=1) as s0p, \
             tc.tile_pool(name="s0ps", bufs=3, space="PSUM") as s0ps, \
             tc.tile_pool(name="s0ss", bufs=1, space="PSUM") as s0ssp, \
             tc.tile_pool(name="s0pb", bufs=1, space="PSUM") as s0pb, \
             tc.tile_pool(name="s0sq", bufs=3) as s0sqp:
            hid_sb = s0p.tile([128, 16, SSH], f32r)
            nc.sync.dma_start(hid_sb[:], hidT.ap().rearrange("o p s -> p o s"))
            a_sb = s0p.tile([128, 16, CTOT], f32r)
            for c0, cw in ((0, 512), (512, 512), (1024, 512), (1536, 576)):
                nc.sync.dma_start(
                    a_sb[:, :, c0:c0 + cw],
                    a_t.ap()[:, :, c0:c0 + cw].rearrange("o p c -> p o c"))

            raw = s0p.tile([128, NCT, SSH], f32)
            ss_hq = s0ssp.tile([1, SSH], f32)
            ss_kv = s0ssp.tile([1, SSH], f32)
            for ct in range(NCT):
                w = 128 if ct < 16 else 64
                ps = s0ps.tile([128, SSH], f32, tag="s0ps")
                for hb in range(16):
                    mm(ps[:w], a_sb[:, hb, ct * 128:ct * 128 + w],
                       hid_sb[:, hb, :], hb == 0, hb == 15)
                nc.vector.tensor_copy(raw[:w, ct, :], ps[:w])
                if ct < 16:
                    sq = s0sqp.tile([128, SSH], f32r, tag="s0sq")
                    nc.scalar.activation(sq[:], ps[:], AF.Square)
                    if ct < 12:
                        mm(ss_hq, ones_col, sq, ct == 0, ct == 11)
                    else:
                        mm(ss_kv, ones_col, sq, ct == 12, ct == 15)

            # rms scale factors: rsqrt(sumsq/D + eps), broadcast to 128 parts
            sq_hq = s0p.tile([1, SSH], f32)
            nc.scalar.activation(sq_hq[:], ss_hq[:], AF.Sqrt,
                                 bias=eps_sb[:], scale=1.0 / Q_LORA)
            rc_hq = s0p.tile([1, SSH], f32r)
            with nc.allow_low_precision(reason="f32r rms scale is fine"):
                nc.vector.reciprocal(rc_hq[:], sq_hq[:])
            sq_kv = s0p.tile([1, SSH], f32)
            nc.scalar.activation(sq_kv[:], ss_kv[:], AF.Sqrt,
                                 bias=eps_sb[:], scale=1.0 / KV_LORA)
            rc_kv = s0p.tile([1, SSH], f32r)
            with nc.allow_low_precision(reason="f32r rms scale is fine"):
                nc.vector.reciprocal(rc_kv[:], sq_kv[:])

            psb_hq = s0pb.tile([128, SSH], f32, tag="s0pb")
            mm(psb_hq, ones_row, rc_hq, True, True)
            bc_hq = s0p.tile([128, SSH], f32)
            nc.scalar.copy(bc_hq[:], psb_hq[:])
            psb_kv = s0pb.tile([128, SSH], f32, tag="s0pb")
            mm(psb_kv, ones_row, rc_kv, True, True)
            bc_kv = s0p.tile([128, SSH], f32)
            nc.scalar.copy(bc_kv[:], psb_kv[:])

            lat = s0p.tile([128, NCT, SSH], f32r)
            for ct in range(12):
                nc.vector.tensor_tensor(lat[:, ct, :], raw[:, ct, :],
                                        bc_hq[:], OP.mult)
            for ct in range(12, 16):
                nc.vector.tensor_tensor(lat[:, ct, :], raw[:, ct, :],
                                        bc_kv[:], OP.mult)
            # k_pe rope (not normalized); rows [0:64) of c-tile 16
            cs_sb = s0p.tile([64, SSH], f32)
            nc.sync.dma_start(cs_sb[:], cosT_sh.ap())
            sn_sb = s0p.tile([64, SSH], f32)
            nc.sync.dma_start(sn_sb[:], sinTs_sh.ap())
            t1 = s0p.tile([64, SSH], f32)
            nc.vector.tensor_tensor(t1[:], raw[0:64, 16, :], cs_sb[:], OP.mult)
            rsw = s0p.tile([64, SSH], f32)
            nc.sync.dma_start(rsw[0:32], raw[32:64, 16, :])
            nc.sync.dma_start(rsw[32:64], raw[0:32, 16, :])
            t2 = s0p.tile([64, SSH], f32)
            nc.vector.tensor_tensor(t2[:], rsw[:], sn_sb[:], OP.mult)
            nc.vector.tensor_tensor(lat[0:64, 16, :], t1[:], t2[:], OP.add)
            nc.sync.dma_start(lat[64:128, 16, :], zer64.ap())
            nc.sync.dma_start(bounce1.ap().rearrange("o p s -> p o s"), lat[:])

        nc.gpsimd.collective_compute(
            "AllGather", OP.bypass, replica_groups=RG,
            ins=[bounce1.ap().opt()], outs=[gath1.ap().opt()])

        # ---------------- stage 1: per-head projections + attention
        with tc.tile_pool(name="s1w", bufs=1) as s1w, \
             tc.tile_pool(name="att", bufs=1) as attp:
            wqb_sb = s1w.tile([128, 12, 384], f32r)
            nc.sync.dma_start(wqb_sb[:], wqbT.ap().rearrange("o p d -> p o d"))
            wkvb_sb = s1w.tile([128, 4, 512], f32r)
            nc.sync.dma_start(wkvb_sb[:], wkvbT.ap().rearrange("o p d -> p o d"))

            kv_sb = s1w.tile([128, 32, SSH], f32r)
            kpe_sb = attp.tile([64, 8, SSH], f32r)
            for r in range(NCORES):
                nc.sync.dma_start(
                    kv_sb[:, r * 4:(r + 1) * 4, :],
                    gath1.ap()[r, 12:16].rearrange("o p s -> p o s"))
                nc.sync.dma_start(kpe_sb[:, r, :], gath1.ap()[r, 16, 0:64, :])

            qn0 = attp.tile([128, S], f32r)
            qt1 = attp.tile([128, S], f32)
            qn1 = attp.tile([128, S], f32r)
            qdst = (qn0, qt1, qn1)
            kn0 = attp.tile([128, S], f32r)
            kn1 = attp.tile([128, S], f32r)
            kn = (kn0, kn1)
            vt = [attp.tile([128, 256], f32r, name=f"vt{tb}")
                  for tb in range(16)]

            with tc.tile_pool(name="hq", bufs=2) as hqp, \
                 tc.tile_pool(name="p1ps", bufs=3, space="PSUM") as p1ps:
                for r in range(NCORES):
                    hq_sb = hqp.tile([128, 12, SSH], f32r, tag="hq")
                    nc.sync.dma_start(
                        hq_sb[:], gath1.ap()[r, 0:12].rearrange("o p s -> p o s"))
                    for m in range(3):
                        ps = p1ps.tile([128, SSH], f32, tag="p1ps")
                        for cc in range(12):
                            mm(ps, wqb_sb[:, cc, m * 128:(m + 1) * 128],
                               hq_sb[:, cc, :], cc == 0, cc == 11)
                        nc.scalar.copy(qdst[m][:, r * SSH:(r + 1) * SSH], ps[:])
                for kh in range(2):
                    for t8 in range(8):
                        ps = p1ps.tile([128, SSH], f32, tag="p1ps")
                        for cc in range(4):
                            mm(ps, wkvb_sb[:, cc, kh * 128:(kh + 1) * 128],
                               kv_sb[:, t8 * 4 + cc, :], cc == 0, cc == 3)
                        nc.scalar.copy(kn[kh][:, t8 * SSH:(t8 + 1) * SSH], ps[:])
                for tb in range(16):
                    ps = p1ps.tile([128, SSH], f32, tag="p1ps")
                    for cc in range(4):
                        mm(ps, kv_sb[:, (tb // 2) * 4 + cc,
                                     (tb % 2) * 128:(tb % 2) * 128 + 128],
                           wkvb_sb[:, cc, 256:512], cc == 0, cc == 3)
                        # lhsT = kvnT chunk [c,t], rhs = v columns of wkv_b'^T
                    nc.scalar.copy(vt[tb][:], ps[:])

            # rope on q (both heads share qt1: rows 0:64 h0, 64:128 h1)
            qt1r = attp.tile([128, S], f32r)
            qr1 = attp.tile([64, S], f32r)
            with tc.tile_pool(name="rope", bufs=1) as rp:
                cos2_sb = rp.tile([128, S], f32)
                nc.sync.dma_start(cos2_sb[:], cosT2.ap())
                sin2_sb = rp.tile([128, S], f32)
                nc.sync.dma_start(sin2_sb[:], sinT2s.ap())
                tmp = rp.tile([128, S], f32)
                for b in (0, 64):
                    nc.sync.dma_start(tmp[b:b + 32], qt1[b + 32:b + 64])
                    nc.sync.dma_start(tmp[b + 32:b + 64], qt1[b:b + 32])
                nc.vector.tensor_tensor(qt1r[:], qt1[:], cos2_sb[:], OP.mult)
                nc.vector.tensor_tensor(tmp[:], tmp[:], sin2_sb[:], OP.mult)
                nc.vector.tensor_tensor(qt1r[:], qt1r[:], tmp[:], OP.add)
                # h1 rope rows to a base-0 tile for use as matmul rhs
                nc.sync.dma_start(qr1[:], qt1r[64:128])

            # attention, streaming over t in chunks of 128
            with tc.tile_pool(name="apss", bufs=2, space="PSUM") as apss, \
                 tc.tile_pool(name="apsx", bufs=2, space="PSUM") as apsx, \
                 tc.tile_pool(name="apsd", bufs=2, space="PSUM") as apsd, \
                 tc.tile_pool(name="apsb", bufs=2, space="PSUM") as apsb, \
                 tc.tile_pool(name="aex", bufs=3) as aexp, \
                 tc.tile_pool(name="asm", bufs=2) as asmp, \
                 tc.tile_pool(name="amk", bufs=2) as amkp, \
                 tc.tile_pool(name="xh", bufs=1) as xhp:
                for h in range(2):
                    qr_h = qt1r if h == 0 else qr1
                    xh = xhp.tile([128, S], f32r, name=f"xh{h}")
                    for sb in range(4):
                        psx = apsx.tile([128, 512], f32, tag="apsx")
                        psd = apsd.tile([1, 512], f32, tag="apsd")
                        for tb in range(16):
                            pss = apss.tile([128, 512], f32, tag="apss")
                            mm(pss, kn[h][:, tb * 128:(tb + 1) * 128],
                               qn0[:, sb * 512:(sb + 1) * 512] if h == 0
                               else qn1[:, sb * 512:(sb + 1) * 512],
                               True, False)
                            mm(pss, kpe_sb[:, tb // 2,
                                           (tb % 2) * 128:(tb % 2) * 128 + 128],
                               qr_h[0:64, sb * 512:(sb + 1) * 512],
                               False, True)
                            if has_mask:
                                mk = amkp.tile([128, 512], f32, tag="amk")
                                nc.sync.dma_start(
                                    mk[:], maskT.ap()[tb * 128:(tb + 1) * 128,
                                                      sb * 512:(sb + 1) * 512])
                                nc.vector.tensor_tensor(pss[:], pss[:], mk[:],
                                                        OP.add)
                            ex = aexp.tile([128, 512], f32r, tag="aex")
                            nc.scalar.activation(ex[:], pss[:], AF.Exp)
                            mm(psx, vt[tb][:, h * 128:(h + 1) * 128], ex,
                               tb == 0, tb == 15)
                            mm(psd, ones_col, ex, tb == 0, tb == 15)
                        rd = asmp.tile([1, 512], f32r, tag="rd")
                        with nc.allow_low_precision(reason="f32r softmax denom"):
                            nc.vector.reciprocal(rd[:], psd[:])
                        psb2 = apsb.tile([128, 512], f32, tag="apsb")
                        mm(psb2, ones_row, rd, True, True)
                        rdb = asmp.tile([128, 512], f32, tag="rdb")
                        nc.vector.tensor_copy(rdb[:], psb2[:])
                        nc.vector.tensor_tensor(
                            xh[:, sb * 512:(sb + 1) * 512], psx[:], rdb[:],
                            OP.mult)
                    nc.sync.dma_start(bounce2.ap()[h], xh[:])

        nc.gpsimd.collective_compute(
            "AllGather", OP.bypass, replica_groups=RG,
            ins=[bounce2.ap().opt()], outs=[gath2.ap().opt()])

        # ---------------- output projection (column-parallel over H)
        with tc.tile_pool(name="wo", bufs=1) as wop, \
             tc.tile_pool(name="wops", bufs=2, space="PSUM") as wops, \
             tc.tile_pool(name="woot", bufs=3) as wootp:
            wot_sb = wop.tile([128, 16, SSH], f32r)
            nc.sync.dma_start(wot_sb[:], woT.ap().rearrange("o p s -> p o s"))
            big_xe = wop.tile([128, 16, S], f32r)
            for k in range(16):
                nc.sync.dma_start(big_xe[:, k, :], gath2.ap()[k])
            for st in range(16):
                pso = wops.tile([128, SSH], f32, tag="wops")
                for k in range(16):
                    mm(pso, big_xe[:, k, st * 128:(st + 1) * 128],
                       wot_sb[:, k, :], k == 0, k == 15)
                ot = wootp.tile([128, SSH], f32, tag="ot")
                nc.scalar.copy(ot[:], pso[:])
                nc.sync.dma_start(out.ap()[st * 128:(st + 1) * 128, :], ot[:])

    nc.compile()
    return nc


def _prep_inputs(hidden_states, cos, sin, attn_mask, wq_a, q_norm_w, wq_b,
                 wkv_a, kv_norm_w, wkv_b, wo, has_mask):
    c = np.ascontiguousarray
    hid = np.asarray(hidden_states, np.float32)[0]          # [S, H]
    hidT = hid.T                                            # [H, S]
    A_T = np.vstack([np.asarray(wq_a, np.float32),
                     np.asarray(wkv_a, np.float32)]).T      # [H, CTOT]
    a_t = c(A_T.reshape(16, 128, CTOT))

    cosT = np.asarray(cos, np.float32).T                    # [64, S]
    sinT = np.asarray(sin, np.float32).T
    sinTs = sinT.copy()
    sinTs[0:32] *= -1.0
    cosT2 = c(np.concatenate([cosT, cosT], 0))              # [128, S]
    sinT2s = c(np.concatenate([sinTs, sinTs], 0))

    wqb = np.asarray(wq_b, np.float32) * np.asarray(q_norm_w, np.float32)[None]
    wqb = wqb * SOFTMAX_SCALE
    wkvb = (np.asarray(wkv_b, np.float32)
            * np.asarray(kv_norm_w, np.float32)[None])
    woT_full = np.asarray(wo, np.float32).T                 # [NH*DV, H]

    qperm = np.r_[0:128, 128:192, 320:384, 192:320]
    kvperm = np.r_[0:128, 256:384, 128:256, 384:512]

    in_maps = []
    for r in range(NCORES):
        m = {
            "hidT": c(hidT[:, r * SSH:(r + 1) * SSH].reshape(16, 128, SSH)),
            "a_t": a_t,
            "cosT_sh": c(cosT[:, r * SSH:(r + 1) * SSH]),
            "sinTs_sh": c(sinTs[:, r * SSH:(r + 1) * SSH]),
            "cosT2": cosT2,
            "sinT2s": sinT2s,
            "wqbT": c(wqb[r * 384:(r + 1) * 384].T[:, qperm]
                      .reshape(12, 128, 384)),
            "wkvbT": c(wkvb[r * 512:(r + 1) * 512].T[:, kvperm]
                       .reshape(4, 128, 512)),
            "woT": c(woT_full[:, r * SSH:(r + 1) * SSH].reshape(16, 128, SSH)),
            "ones_a": np.ones((128, 1), np.float32),
            "ones_b": np.ones((1, 128), np.float32),
            "zer64": np.zeros((64, SSH), np.float32),
        }
        if has_mask:
            m["maskT"] = c(np.asarray(attn_mask, np.float32).T)
        in_maps.append(m)
    return in_maps


def kernel(**inputs):
    from concourse.bass_utils import run_bass_kernel_spmd

    has_mask = bool(np.any(np.asarray(inputs["attn_mask"])))
    if has_mask not in _CACHE:
        _CACHE[has_mask] = _build(has_mask)
    nc = _CACHE[has_mask]

    in_maps = _prep_inputs(has_mask=has_mask, **inputs)
    res = run_bass_kernel_spmd(nc, in_maps, list(range(NCORES))).results
    full = np.concatenate([res[r]["out"] for r in range(NCORES)], axis=1)
    return full.reshape(B, S, H).astype(np.float32)



# revision 7
# speedup vs baseline: 1.6413x; 1.6413x over previous
"""MLA (DeepSeek-style multi-head latent attention) Bass kernel for 8 trn2 NeuronCores.

v2 design, bf16 compute:
- Stage 0 (sequence-sharded, 256 tokens/core): latents in [c, s] layout, kv
  c-tiles first so the small kv AllGather is issued early and overlaps the q
  c-tile compute; the bigger q AllGather overlaps the k/v projections.
- Stage 1 (tensor-parallel, 2 heads/core): k_nope/v from gathered kv latents,
  q from gathered q latents; rope on q/k_pe with host-folded signs.
- Attention per head with sb-paired N=512 streams (stationary weights reused
  across the pair), softmax without max-subtraction, denominator via
  ones-matmul, reciprocal broadcast on GpSimd.
- Output: AllToAll redistributes x from head-sharding to token-sharding, then
  each core computes its 256 output rows against the full wo (column streams),
  so there is no AllReduce and no 16 MB gather on the critical path.

All matmul operands bf16 (fp32 PSUM accumulation); norms/softmax stats fp32.
Host-side (free) preprocessing: weight transposes/permutations, norm and
softmax-scale folding, rope sign folding, bf16 casts.
"""

import math
import sys

import numpy as np

for _p in ("/opt/trn_rl_repo", "/root/.axon_site/_ro/trn_rl_repo"):
    if _p not in sys.path:
        sys.path.append(_p)

B, S, H = 1, 2048, 2048
NH = 16
Q_LORA, KV_LORA = 1536, 512
D_NOPE, D_ROPE, D_V = 128, 64, 128
D_QK = D_NOPE + D_ROPE
ROPE_FACTOR, MSCALE = 4.0, 1.0
SOFTMAX_SCALE = D_QK ** -0.5 * (0.1 * MSCALE * math.log(ROPE_FACTOR) + 1.0) ** 2
EPS = 1e-6

NCORES = 8
SSH = S // NCORES          # 256 tokens per core in stage 0

_CACHE = {}


def _build(has_mask: bool):
    import concourse.bacc as bacc
    import concourse.mybir as mybir
    import concourse.tile as tile

    bf = mybir.dt.bfloat16
    f32 = mybir.dt.float32
    f32r = mybir.dt.float32r
    AF = mybir.ActivationFunctionType
    OP = mybir.AluOpType

    nc = bacc.Bacc("TRN2", target_bir_lowering=False, debug=False,
                   num_devices=NCORES)

    hidT = nc.dram_tensor("hidT", [16, 128, SSH], bf, kind="ExternalInput")
    a_t = nc.dram_tensor("a_t", [16, 128, 2112], bf, kind="ExternalInput")
    cosT_sh = nc.dram_tensor("cosT_sh", [64, SSH], f32, kind="ExternalInput")
    sinTs_sh = nc.dram_tensor("sinTs_sh", [64, SSH], f32, kind="ExternalInput")
    cosT2 = nc.dram_tensor("cosT2", [128, S], f32, kind="ExternalInput")
    sinT2s = nc.dram_tensor("sinT2s", [128, S], f32, kind="ExternalInput")
    wqbT = nc.dram_tensor("wqbT", [12, 128, 384], bf, kind="ExternalInput")
    wkvbT = nc.dram_tensor("wkvbT", [4, 128, 512], bf, kind="ExternalInput")
    woT = nc.dram_tensor("woT", [16, 128, S], bf, kind="ExternalInput")
    ones_bf = nc.dram_tensor("ones_bf", [128, 1], bf, kind="ExternalInput")
    ones_fr = nc.dram_tensor("ones_fr", [128, 1], f32r, kind="ExternalInput")
    onesr_fr = nc.dram_tensor("onesr_fr", [1, 128], f32r, kind="ExternalInput")
    if has_mask:
        maskT = nc.dram_tensor("maskT", [S, S], f32, kind="ExternalInput")
    out = nc.dram_tensor("out", [SSH, S], f32, kind="ExternalOutput")

    bounce_kv = nc.dram_tensor("bounce_kv", [5, 128, SSH], bf)
    gath_kv = nc.dram_tensor("gath_kv", [NCORES, 5, 128, SSH], bf,
                             addr_space="Shared")
    bounce_q = nc.dram_tensor("bounce_q", [12, 128, SSH], bf)
    gath_q = nc.dram_tensor("gath_q", [NCORES, 12, 128, SSH], bf,
                            addr_space="Shared")
    bounce_x = nc.dram_tensor("bounce_x", [NCORES, 2, 128, SSH], bf)
    gath_x = nc.dram_tensor("gath_x", [NCORES, 2, 128, SSH], bf)

    RG = [list(range(NCORES))]

    def mm(ps, lhsT, rhs, start, stop):
        nc.tensor.matmul(ps, lhsT, rhs, start=start, stop=stop)

    from contextlib import ExitStack
    with tile.TileContext(nc) as tc, ExitStack() as _st:
        constp = _st.enter_context(tc.tile_pool(name="const", bufs=1))
        ones_c_bf = constp.tile([128, 1], bf)
        nc.sync.dma_start(ones_c_bf[:], ones_bf.ap())
        ones_c_fr = constp.tile([128, 1], f32r)
        nc.sync.dma_start(ones_c_fr[:], ones_fr.ap())
        ones_r_fr = constp.tile([1, 128], f32r)
        nc.sync.dma_start(ones_r_fr[:], onesr_fr.ap())
        eps_sb = constp.tile([1, 1], f32)
        nc.any.memset(eps_sb[:], EPS)

        # ---------------- stage 0: latents for own 256 tokens, [c, s] layout
        with tc.tile_pool(name="s0", bufs=1) as s0p, \
             tc.tile_pool(name="s0ps", bufs=3, space="PSUM") as s0ps, \
             tc.tile_pool(name="s0ss", bufs=1, space="PSUM") as s0ssp, \
             tc.tile_pool(name="s0pb", bufs=2, space="PSUM") as s0pb, \
             tc.tile_pool(name="s0sq", bufs=3) as s0sqp:
            hid_sb = s0p.tile([128, 16, SSH], bf)
            nc.sync.dma_start(hid_sb[:], hidT.ap().rearrange("o p s -> p o s"))
            a_kv_sb = s0p.tile([128, 16, 576], bf)
            nc.sync.dma_start(
                a_kv_sb[:], a_t.ap()[:, :, 1536:2112].rearrange("o p c -> p o c"))
            a_q_sb = s0p.tile([128, 16, 1536], bf)
            for c0 in (0, 512, 1024):
                nc.sync.dma_start(
                    a_q_sb[:, :, c0:c0 + 512],
                    a_t.ap()[:, :, c0:c0 + 512].rearrange("o p c -> p o c"))

            # kv c-tiles (4 full + kpe 64)
            raw_kv = s0p.tile([128, 5, SSH], f32)
            ss_kv = s0ssp.tile([1, SSH], f32)
            for i in range(5):
                w = 128 if i < 4 else 64
                ps = s0ps.tile([128, SSH], f32, tag="s0ps")
                for hb in range(16):
                    mm(ps[:w], a_kv_sb[:, hb, i * 128:i * 128 + w],
                       hid_sb[:, hb, :], hb == 0, hb == 15)
                nc.vector.tensor_copy(raw_kv[:w, i, :], ps[:w])
                if i < 4:
                    sq = s0sqp.tile([128, SSH], f32r, tag="s0sq")
                    nc.scalar.activation(sq[:], ps[:], AF.Square)
                    mm(ss_kv, ones_c_fr, sq, i == 0, i == 3)

            sq_kv = s0p.tile([1, SSH], f32)
            nc.scalar.activation(sq_kv[:], ss_kv[:], AF.Sqrt,
                                 bias=eps_sb[:], scale=1.0 / KV_LORA)
            rc_kv = s0p.tile([1, SSH], f32r)
            with nc.allow_low_precision(reason="f32r rms scale is fine"):
                nc.vector.reciprocal(rc_kv[:], sq_kv[:])
            psb_kv = s0pb.tile([128, SSH], f32, tag="s0pb")
            mm(psb_kv, ones_r_fr, rc_kv, True, True)
            bc_kv = s0p.tile([128, SSH], f32)
            nc.scalar.copy(bc_kv[:], psb_kv[:])

            lat_kv = s0p.tile([128, 5, SSH], bf)
            for i in range(4):
                nc.vector.tensor_tensor(lat_kv[:, i, :], raw_kv[:, i, :],
                                        bc_kv[:], OP.mult)
            # k_pe rope (not normalized); rows [0:64) of tile 4
            cs_sb = s0p.tile([64, SSH], f32)
            nc.sync.dma_start(cs_sb[:], cosT_sh.ap())
            sn_sb = s0p.tile([64, SSH], f32)
            nc.sync.dma_start(sn_sb[:], sinTs_sh.ap())
            rsw = s0p.tile([64, SSH], f32)
            nc.sync.dma_start(rsw[0:32], raw_kv[32:64, 4, :])
            nc.sync.dma_start(rsw[32:64], raw_kv[0:32, 4, :])
            t1 = s0p.tile([64, SSH], f32)
            nc.vector.tensor_tensor(t1[:], raw_kv[0:64, 4, :], cs_sb[:],
                                    OP.mult)
            nc.vector.tensor_tensor(rsw[:], rsw[:], sn_sb[:], OP.mult)
            nc.vector.tensor_tensor(lat_kv[0:64, 4, :], t1[:], rsw[:], OP.add)
            nc.any.memset(lat_kv[64:128, 4, :], 0.0)
            nc.sync.dma_start(bounce_kv.ap().rearrange("o p s -> p o s"),
                              lat_kv[:])

            nc.gpsimd.collective_compute(
                "AllGather", OP.bypass, replica_groups=RG,
                ins=[bounce_kv.ap().opt()], outs=[gath_kv.ap().opt()])

            # q c-tiles
            raw_q = s0p.tile([128, 12, SSH], f32)
            ss_hq = s0ssp.tile([1, SSH], f32)
            for ct in range(12):
                ps = s0ps.tile([128, SSH], f32, tag="s0ps")
                for hb in range(16):
                    mm(ps, a_q_sb[:, hb, ct * 128:(ct + 1) * 128],
                       hid_sb[:, hb, :], hb == 0, hb == 15)
                nc.vector.tensor_copy(raw_q[:, ct, :], ps[:])
                sq = s0sqp.tile([128, SSH], f32r, tag="s0sq")
                nc.scalar.activation(sq[:], ps[:], AF.Square)
                mm(ss_hq, ones_c_fr, sq, ct == 0, ct == 11)

            sq_hq = s0p.tile([1, SSH], f32)
            nc.scalar.activation(sq_hq[:], ss_hq[:], AF.Sqrt,
                                 bias=eps_sb[:], scale=1.0 / Q_LORA)
            rc_hq = s0p.tile([1, SSH], f32r)
            with nc.allow_low_precision(reason="f32r rms scale is fine"):
                nc.vector.reciprocal(rc_hq[:], sq_hq[:])
            psb_hq = s0pb.tile([128, SSH], f32, tag="s0pb")
            mm(psb_hq, ones_r_fr, rc_hq, True, True)
            bc_hq = s0p.tile([128, SSH], f32)
            nc.scalar.copy(bc_hq[:], psb_hq[:])

            lat_q = s0p.tile([128, 12, SSH], bf)
            for ct in range(12):
                nc.vector.tensor_tensor(lat_q[:, ct, :], raw_q[:, ct, :],
                                        bc_hq[:], OP.mult)
            nc.sync.dma_start(bounce_q.ap().rearrange("o p s -> p o s"),
                              lat_q[:])

            nc.gpsimd.collective_compute(
                "AllGather", OP.bypass, replica_groups=RG,
                ins=[bounce_q.ap().opt()], outs=[gath_q.ap().opt()])

        # ---------------- stage 1: per-head projections + attention
        with tc.tile_pool(name="s1w", bufs=1) as s1w, \
             tc.tile_pool(name="att", bufs=1) as attp:
            wqb_sb = s1w.tile([128, 12, 384], bf)
            nc.sync.dma_start(wqb_sb[:], wqbT.ap().rearrange("o p d -> p o d"))
            wkvb_sb = s1w.tile([128, 4, 512], bf)
            nc.sync.dma_start(wkvb_sb[:], wkvbT.ap().rearrange("o p d -> p o d"))

            kv_sb = s1w.tile([128, 4, S], bf)
            kpe_sb = attp.tile([64, S], bf)
            for r in range(NCORES):
                nc.sync.dma_start(
                    kv_sb[:, :, r * SSH:(r + 1) * SSH],
                    gath_kv.ap()[r, 0:4].rearrange("o p s -> p o s"))
                nc.sync.dma_start(kpe_sb[:, r * SSH:(r + 1) * SSH],
                                  gath_kv.ap()[r, 4, 0:64, :])

            kn_sb = s1w.tile([128, 2, S], bf)
            vt_sb = s1w.tile([128, 16, 256], bf)
            qn0 = attp.tile([128, S], bf)
            qn1 = attp.tile([128, S], bf)
            qt1 = attp.tile([128, S], f32)

            with tc.tile_pool(name="p1ps", bufs=3, space="PSUM") as p1ps, \
                 tc.tile_pool(name="p1psv", bufs=2, space="PSUM") as p1psv:
                # k_nope for the 2 own heads
                for kh in range(2):
                    for tch in range(4):
                        ps = p1ps.tile([128, 512], f32, tag="p1ps")
                        for cc in range(4):
                            mm(ps, wkvb_sb[:, cc, kh * 128:(kh + 1) * 128],
                               kv_sb[:, cc, tch * 512:(tch + 1) * 512],
                               cc == 0, cc == 3)
                        nc.scalar.copy(kn_sb[:, kh, tch * 512:(tch + 1) * 512],
                                       ps[:])
                # v^T tiles [t, 2*128]
                for tb in range(16):
                    ps = p1psv.tile([128, 256], f32, tag="p1psv")
                    for cc in range(4):
                        mm(ps, kv_sb[:, cc, tb * 128:(tb + 1) * 128],
                           wkvb_sb[:, cc, 256:512], cc == 0, cc == 3)
                    nc.vector.tensor_copy(vt_sb[:, tb, :], ps[:])

                # q projections (needs the q AllGather)
                with tc.tile_pool(name="hq", bufs=1) as hqp:
                    hq_sb = hqp.tile([128, 12, S], bf)
                    for r in range(NCORES):
                        nc.sync.dma_start(
                            hq_sb[:, :, r * SSH:(r + 1) * SSH],
                            gath_q.ap()[r].rearrange("o p s -> p o s"))
                    qdst = (qn0, qt1, qn1)
                    for m in range(3):
                        for tch in range(4):
                            ps = p1ps.tile([128, 512], f32, tag="p1ps")
                            for cc in range(12):
                                mm(ps, wqb_sb[:, cc, m * 128:(m + 1) * 128],
                                   hq_sb[:, cc, tch * 512:(tch + 1) * 512],
                                   cc == 0, cc == 11)
                            if m == 1:
                                nc.vector.tensor_copy(
                                    qt1[:, tch * 512:(tch + 1) * 512], ps[:])
                            else:
                                nc.scalar.copy(
                                    qdst[m][:, tch * 512:(tch + 1) * 512],
                                    ps[:])

            # rope on q (both heads share qt1: rows 0:64 h0, 64:128 h1)
            qrb = attp.tile([128, S], bf)
            qr1 = attp.tile([64, S], bf)
            with tc.tile_pool(name="rope", bufs=1) as rp:
                cos2_sb = rp.tile([128, S], f32)
                nc.sync.dma_start(cos2_sb[:], cosT2.ap())
                sin2_sb = rp.tile([128, S], f32)
                nc.sync.dma_start(sin2_sb[:], sinT2s.ap())
                tmp = rp.tile([128, S], f32)
                for b in (0, 64):
                    nc.sync.dma_start(tmp[b:b + 32], qt1[b + 32:b + 64])
                    nc.sync.dma_start(tmp[b + 32:b + 64], qt1[b:b + 32])
                nc.vector.tensor_tensor(qt1[:], qt1[:], cos2_sb[:], OP.mult)
                nc.vector.tensor_tensor(tmp[:], tmp[:], sin2_sb[:], OP.mult)
                nc.vector.tensor_tensor(qrb[:], qt1[:], tmp[:], OP.add)
                nc.sync.dma_start(qr1[:], qrb[64:128])

            # full wo, loaded during attention
            wo_sb = s1w.tile([128, 16, S], bf)
            nc.sync.dma_start(wo_sb[:], woT.ap().rearrange("o p s -> p o s"))

            # attention, 2 heads, sb-paired 512-token column chunks
            with tc.tile_pool(name="apss", bufs=4, space="PSUM") as apss, \
                 tc.tile_pool(name="apsx", bufs=2, space="PSUM") as apsx, \
                 tc.tile_pool(name="apsd", bufs=2, space="PSUM") as apsd, \
                 tc.tile_pool(name="aex", bufs=4) as aexp, \
                 tc.tile_pool(name="asm", bufs=4) as asmp, \
                 tc.tile_pool(name="amk", bufs=4) as amkp, \
                 tc.tile_pool(name="xh", bufs=1) as xhp:
                for h in range(2):
                    qn_h = qn0 if h == 0 else qn1
                    qr_h = qrb if h == 0 else qr1
                    xh = xhp.tile([128, S], bf, name=f"xh{h}")
                    for sbp in range(2):
                        sA = 2 * sbp
                        sB = sA + 1
                        psx = [apsx.tile([128, 512], f32, tag="apsx",
                                         name=f"psx{j}")
                               for j in range(2)]
                        psd = [apsd.tile([1, 512], f32, tag="apsd",
                                         name=f"psd{j}")
                               for j in range(2)]
                        for tb in range(16):
                            pss = [apss.tile([128, 512], f32, tag="apss",
                                             name=f"pss{j}")
                                   for j in range(2)]
                            for j, sb in enumerate((sA, sB)):
                                mm(pss[j], kn_sb[:, h, tb * 128:(tb + 1) * 128],
                                   qn_h[:, sb * 512:(sb + 1) * 512],
                                   True, False)
                            for j, sb in enumerate((sA, sB)):
                                mm(pss[j], kpe_sb[:, tb * 128:(tb + 1) * 128],
                                   qr_h[0:64, sb * 512:(sb + 1) * 512],
                                   False, True)
                            ex = []
                            for j, sb in enumerate((sA, sB)):
                                if has_mask:
                                    mk = amkp.tile([128, 512], f32, tag="amk")
                                    nc.sync.dma_start(
                                        mk[:],
                                        maskT.ap()[tb * 128:(tb + 1) * 128,
                                                   sb * 512:(sb + 1) * 512])
                                    nc.vector.tensor_tensor(
                                        pss[j][:], pss[j][:], mk[:], OP.add)
                                e = aexp.tile([128, 512], bf, tag="aex",
                                              name=f"ex{j}")
                                nc.scalar.activation(e[:], pss[j][:], AF.Exp)
                                ex.append(e)
                            for j in range(2):
                                mm(psx[j], vt_sb[:, tb, h * 128:(h + 1) * 128],
                                   ex[j], tb == 0, tb == 15)
                            for j in range(2):
                                mm(psd[j], ones_c_bf, ex[j],
                                   tb == 0, tb == 15)
                        for j, sb in enumerate((sA, sB)):
                            rd = asmp.tile([1, 512], f32, tag="rd")
                            with nc.allow_low_precision(
                                    reason="softmax denom reciprocal"):
                                nc.vector.reciprocal(rd[:], psd[j][:])
                            bcd = asmp.tile([128, 512], f32, tag="bcd")
                            nc.gpsimd.partition_broadcast(bcd[:], rd[:])
                            nc.vector.tensor_tensor(
                                xh[:, sb * 512:(sb + 1) * 512], psx[j][:],
                                bcd[:], OP.mult)
                    for j in range(NCORES):
                        nc.sync.dma_start(
                            bounce_x.ap()[j, h],
                            xh[:, j * SSH:(j + 1) * SSH])

            nc.gpsimd.collective_compute(
                "AllToAll", OP.bypass, replica_groups=RG,
                ins=[bounce_x.ap().opt()], outs=[gath_x.ap().opt()])

            # ---------------- output projection (token-sharded rows)
            with tc.tile_pool(name="wops", bufs=2, space="PSUM") as wops, \
                 tc.tile_pool(name="woot", bufs=3) as wootp:
                xg_sb = attp.tile([128, 16, SSH], bf)
                for r in range(NCORES):
                    for h in range(2):
                        nc.sync.dma_start(xg_sb[:, r * 2 + h, :],
                                          gath_x.ap()[r, h])
                for st2 in range(2):
                    for hc in range(4):
                        ps = wops.tile([128, 512], f32, tag="wops")
                        for cc in range(16):
                            mm(ps, xg_sb[:, cc, st2 * 128:(st2 + 1) * 128],
                               wo_sb[:, cc, hc * 512:(hc + 1) * 512],
                               cc == 0, cc == 15)
                        ot = wootp.tile([128, 512], f32, tag="ot")
                        nc.scalar.copy(ot[:], ps[:])
                        nc.sync.dma_start(
                            out.ap()[st2 * 128:(st2 + 1) * 128,
                                     hc * 512:(hc + 1) * 512], ot[:])

    nc.compile()
    return nc


def _prep_inputs(hidden_states, cos, sin, attn_mask, wq_a, q_norm_w, wq_b,
                 wkv_a, kv_norm_w, wkv_b, wo, has_mask):
    import ml_dtypes
    bf16 = ml_dtypes.bfloat16
    c = np.ascontiguousarray

    hid = np.asarray(hidden_states, np.float32)[0]          # [S, H]
    hidT = hid.T.astype(bf16)                               # [H, S]
    A_T = np.vstack([np.asarray(wq_a, np.float32),
                     np.asarray(wkv_a, np.float32)]).T      # [H, 2112]
    a_t = c(A_T.astype(bf16).reshape(16, 128, 2112))

    cosT = np.asarray(cos, np.float32).T                    # [64, S]
    sinT = np.asarray(sin, np.float32).T
    sinTs = sinT.copy()
    sinTs[0:32] *= -1.0
    cosT2 = c(np.concatenate([cosT, cosT], 0))              # [128, S]
    sinT2s = c(np.concatenate([sinTs, sinTs], 0))

    wqb = np.asarray(wq_b, np.float32) * np.asarray(q_norm_w, np.float32)[None]
    wqb = wqb * SOFTMAX_SCALE
    wkvb = (np.asarray(wkv_b, np.float32)
            * np.asarray(kv_norm_w, np.float32)[None])
    woT_full = c(np.asarray(wo, np.float32).T.astype(bf16)
                 .reshape(16, 128, S))                      # [NH*DV, H]

    qperm = np.r_[0:128, 128:192, 320:384, 192:320]
    kvperm = np.r_[0:128, 256:384, 128:256, 384:512]

    in_maps = []
    for r in range(NCORES):
        m = {
            "hidT": c(hidT[:, r * SSH:(r + 1) * SSH].reshape(16, 128, SSH)),
            "a_t": a_t,
            "cosT_sh": c(cosT[:, r * SSH:(r + 1) * SSH]),
            "sinTs_sh": c(sinTs[:, r * SSH:(r + 1) * SSH]),
            "cosT2": cosT2,
            "sinT2s": sinT2s,
            "wqbT": c(wqb[r * 384:(r + 1) * 384].T[:, qperm]
                      .astype(bf16).reshape(12, 128, 384)),
            "wkvbT": c(wkvb[r * 512:(r + 1) * 512].T[:, kvperm]
                       .astype(bf16).reshape(4, 128, 512)),
            "woT": woT_full,
            "ones_bf": np.ones((128, 1), bf16),
            "ones_fr": np.ones((128, 1), np.float32),
            "onesr_fr": np.ones((1, 128), np.float32),
        }
        if has_mask:
            m["maskT"] = c(np.asarray(attn_mask, np.float32).T)
        in_maps.append(m)
    return in_maps


def assemble(res):
    full = np.concatenate([np.asarray(res[r]["out"], np.float32)
                           for r in range(NCORES)], axis=0)
    return full.reshape(B, S, H)


def kernel(**inputs):
    from concourse.bass_utils import run_bass_kernel_spmd

    has_mask = bool(np.any(np.asarray(inputs["attn_mask"])))
    if has_mask not in _CACHE:
        _CACHE[has_mask] = _build(has_mask)
    nc = _CACHE[has_mask]

    in_maps = _prep_inputs(has_mask=has_mask, **inputs)
    res = run_bass_kernel_spmd(nc, in_maps, list(range(NCORES))).results
    return assemble(res)
